# revision 7
# baseline (speedup 1.0000x reference)
"""Diversity7 loss kernel for Trainium2 (8 NeuronCores, Bass/Tile).

Math (per batch row b):
  p_m   = softmax(x_m / T)                          m = 0..6, C = 1000 classes
  v_m   = (p_m - mean(p_m)) / ||p_m - mean(p_m)||   (mean(p_m) = 1/C exactly)
  q_b   = || sum_m v_m ||^2
  loss  = SCALE * mean_b((q_b - M) / 2)

Device-side restructuring (all f32):
  e    = exp(x/T)            (ACT pass, accum_out gives Se = sum e)
  S2   = sum_c e^2           (uncentered; split between ACT Square+accum and
                              DVE affine_mul_reduce(scale=1,bias=0) so neither
                              engine saturates, and no Se->dev2 serialization)
  dev2 = S2 - Se^2/C         (f32 smalls; cancellation costs ~1e-3 rel on
                              dev2 which is far inside the 2e-2 budget)
  g    = rsqrt(dev2) via magic-constant seed + 2 Newton steps (DVE-only)
  h    = -g*Se/C;  s = sum_m (g_m*e_m + h_m)   (fused affine_then_add chain)
  q    = sum_c s^2           (DVE affine_mul_reduce early row-tiles, ACT
                              Square+accum late ones where ACT idles)
Host finishes in f64: loss = SCALE * mean((q-7)/2).

Sharding: data-parallel over batch. 8 cores x 512 rows; each core sees
[512,1000] slices of the 7 logit tensors and emits q for its rows as [128,4]
(partition p, row-tile rt) -> global row = core*512 + rt*128 + p.
`targets` is accepted and ignored (unused by the reference loss).
"""

import sys

import numpy as np

if "/opt/trn_rl_repo" not in sys.path:
    sys.path.insert(0, "/opt/trn_rl_repo")

import concourse.bass as bass
import concourse.tile as tile
from concourse import bacc, mybir
from concourse.bass_utils import run_bass_kernel_spmd


def _patch_act_tables() -> None:
    """Make Exp/Square resolve only via natural_log_exp_and_others so the
    kernel needs exactly one ACT table load (the default first-fit choice
    thrashes table sets, ~1.3us per switch)."""
    import concourse.hw_specs as hw_specs

    if getattr(hw_specs, "_diversity7_patched", False):
        return
    orig = hw_specs.get_activation_tables

    def patched(module_arch):
        tables = orig(module_arch)
        keep = "natural_log_exp_and_others"
        if keep in tables:
            only = {
                mybir.ActivationFunctionType.Exp,
                mybir.ActivationFunctionType.Ln,
                mybir.ActivationFunctionType.Square,
            }
            for name, funcs in tables.items():
                if name != keep:
                    funcs -= only
        return tables

    hw_specs.get_activation_tables = patched
    bacc.get_activation_tables = patched
    hw_specs._diversity7_patched = True


T = 20.0
SCALE = 0.3
C = 1000
M = 7
N_CORES = 8
ROWS_PER_CORE = 512
RT = ROWS_PER_CORE // 128  # row-tiles per core

# Engine balance: how many of the 7 S2 (sum e^2) units run on ACT (Square+
# accum) vs DVE (affine_mul_reduce).  First row-tile keeps ACT light so the
# pipeline fills fast.
ACT_S2_COUNT = (2, 3, 3, 3)
# q = sum s^2 placement per row-tile: DVE early (ACT still busy with exps),
# ACT late (ACT idles during the tail while DVE finishes chains).
Q_ON_ACT = (False, False, True, True)

F32 = mybir.dt.float32
I32 = mybir.dt.int32
AF = mybir.ActivationFunctionType
ALU = mybir.AluOpType


def _build_program() -> bass.Bass:
    _patch_act_tables()
    nc = bacc.Bacc()
    xs = [
        nc.declare_dram_parameter(f"x{m}", [ROWS_PER_CORE, C], F32, isOutput=False)
        for m in range(M)
    ]
    q_out = nc.declare_dram_parameter("q_out", [128, RT], F32, isOutput=True)

    with tile.TileContext(nc) as tc:
        with (
            tc.tile_pool(name="xp", bufs=16) as xp,
            tc.tile_pool(name="ep", bufs=1) as ep,
            tc.tile_pool(name="trp", bufs=3) as trp,
            tc.tile_pool(name="sp", bufs=3) as sp,
            tc.tile_pool(name="smp", bufs=1) as smp,
            tc.tile_pool(name="qp", bufs=1) as qp,
        ):
            q = qp.tile([128, RT], F32)
            # Constant [128,M] tiles so the whole smalls block runs as GpSimd
            # tensor_tensor ops (GPS ts is ~1.4us but GPS tt is ~240ns; DVE
            # is the bottleneck engine so smalls move off it entirely).
            magic_t = smp.tile([128, M], I32, tag="magic_t")
            nc.vector.memset(magic_t[:], 0x5F3759DF)
            one_t = smp.tile([128, M], I32, tag="one_t")
            nc.vector.memset(one_t[:], 1)
            neg_invC = smp.tile([128, M], F32, tag="neg_invC")
            nc.vector.memset(neg_invC[:], -1.0 / C)
            neg_half = smp.tile([128, M], F32, tag="neg_half")
            nc.vector.memset(neg_half[:], -0.5)
            three_half = smp.tile([128, M], F32, tag="three_half")
            nc.vector.memset(three_half[:], 1.5)

            def phase1(rt: int):
                """DMA + exp(+Se accum) + S2 units for row-tile rt."""
                n_act = ACT_S2_COUNT[rt]
                Se = smp.tile([128, M], F32, tag="Se", bufs=2, name=f"Se{rt}")
                S2 = smp.tile([128, M], F32, tag="S2", bufs=2, name=f"S2{rt}")
                es: list[bass.AP] = []
                for m in range(M):
                    k = rt * M + m
                    x = xp.tile([128, C], F32, tag="x", name=f"x_{k}")
                    nc.sync.dma_start(x[:], xs[m][rt * 128 : (rt + 1) * 128, :])
                    e = ep.tile([128, C], F32, tag=f"e{m}", bufs=2, name=f"e_{k}")
                    nc.scalar.activation(
                        e[:], x[:], AF.Exp, bias=0.0, scale=1.0 / T,
                        accum_out=Se[:, m : m + 1],
                    )
                    trash = trp.tile([128, C], F32, tag="trash", name=f"tr_{k}")
                    if m >= M - n_act:
                        # S2 = sum e^2 on ACT (Square, no bias -> no Se dep)
                        nc.scalar.activation(
                            trash[:], e[:], AF.Square, bias=0.0, scale=1.0,
                            accum_out=S2[:, m : m + 1],
                        )
                    else:
                        # S2 = sum (1*e+0)*e on DVE
                        nc.vector.affine_mul_reduce(
                            out=trash[:], accum_out=S2[:, m : m + 1],
                            in0=e[:], in1=e[:], scale=1.0, bias=0.0,
                        )
                    es.append(e)
                return Se, S2, es

            def phase2_3(rt: int, Se, S2, es: list[bass.AP]):
                """Smalls (dev2, rsqrt, g, h) on GpSimd, then chain + q."""
                def gtile(nm, dt=F32, b=2):
                    return smp.tile([128, M], dt, tag=f"sm_{nm}",
                                    bufs=b, name=f"sm_{nm}_{rt}")
                tt = nc.gpsimd.tensor_tensor
                # dev2 = S2 - Se^2/C
                Se2 = gtile("Se2_")
                tt(Se2[:], Se[:], Se[:], ALU.mult)
                m1 = gtile("m1_")
                tt(m1[:], Se2[:], neg_invC[:], ALU.mult)
                dev2 = gtile("dev2_")
                tt(dev2[:], m1[:], S2[:], ALU.add)
                # rsqrt via magic seed + 2 Newton steps (seed rel err 3.4e-3,
                # after 2 steps ~4e-10 -- exact at f32 for this tolerance)
                # Pool can't do 32-bit shifts; this one op stays on DVE.
                half_i = gtile("hi_", I32)
                nc.vector.tensor_scalar(
                    half_i[:], dev2[:].bitcast(I32), one_t[:, 0:1], None,
                    op0=ALU.logical_shift_right,
                )
                seed_i = gtile("si_", I32)
                tt(seed_i[:], magic_t[:], half_i[:], ALU.subtract)
                y = seed_i[:].bitcast(F32)
                for it in range(2):
                    ysq = gtile(f"ysq{it}_")
                    tt(ysq[:], y, y, ALU.mult)
                    zy = gtile(f"zy{it}_")
                    tt(zy[:], dev2[:], ysq[:], ALU.mult)
                    zh = gtile(f"zh{it}_")
                    tt(zh[:], zy[:], neg_half[:], ALU.mult)
                    nrc = gtile(f"nrc{it}_")
                    tt(nrc[:], zh[:], three_half[:], ALU.add)
                    yn = gtile(f"invr{it}_")
                    tt(yn[:], y, nrc[:], ALU.mult)
                    y = yn[:]
                g = y
                hs = gtile("hs_")
                tt(hs[:], Se[:], y, ALU.mult)
                h = gtile("h_")
                tt(h[:], hs[:], neg_invC[:], ALU.mult)

                # s = sum_m (g_m*e_m + h_m) via fused affine_then_add chain
                s_prev = None
                for m in range(M):
                    s_new = sp.tile([128, C], F32, tag="s", name=f"s{rt}_{m}")
                    if m == 0:
                        nc.vector.tensor_scalar(
                            s_new[:], es[0][:], g[0:128, 0:1], h[:, 0:1],
                            op0=ALU.mult, op1=ALU.add,
                        )
                    else:
                        nc.vector.affine_then_add(
                            s_new[:], es[m][:], s_prev[:], g[0:128, m : m + 1],
                            h[:, m : m + 1],
                        )
                    s_prev = s_new
                trash2 = trp.tile([128, C], F32, tag="trash", name=f"tr2_{rt}")
                if Q_ON_ACT[rt]:
                    nc.scalar.activation(
                        trash2[:], s_prev[:], AF.Square, bias=0.0, scale=1.0,
                        accum_out=q[:, rt : rt + 1],
                    )
                else:
                    nc.vector.affine_mul_reduce(
                        out=trash2[:], accum_out=q[:, rt : rt + 1],
                        in0=s_prev[:], in1=s_prev[:], scale=1.0, bias=0.0,
                    )

            # Software pipeline: emit row-tile rt+1's phase 1 BEFORE row-tile
            # rt's scalar math + chain, so the (FIFO) engine queues always
            # have ready phase-1 work at row-tile boundaries.
            DEPTH = 1
            pending = []
            for rt in range(RT):
                pending.append((rt, *phase1(rt)))
                if len(pending) > DEPTH:
                    phase2_3(*pending.pop(0))
            for args in pending:
                phase2_3(*args)
            nc.sync.dma_start(q_out[:], q[:])
    return nc


_NC_CACHE: bass.Bass | None = None


def _get_program() -> bass.Bass:
    global _NC_CACHE
    if _NC_CACHE is None:
        nc = _build_program()
        nc.finalize()
        _NC_CACHE = nc
    return _NC_CACHE


def run_device_part(inputs: dict[str, np.ndarray], **run_kwargs):
    """Run the bass kernel; returns (q_all [4096] f64 row-major, results)."""
    nc = _get_program()
    core_ids = list(range(N_CORES))
    in_maps = []
    for c in range(N_CORES):
        lo, hi = c * ROWS_PER_CORE, (c + 1) * ROWS_PER_CORE
        im = {
            f"x{m}": np.ascontiguousarray(
                inputs[f"outputs{m + 1}"][lo:hi], dtype=np.float32
            )
            for m in range(M)
        }
        in_maps.append(im)
    res = run_bass_kernel_spmd(nc, in_maps, core_ids, **run_kwargs)
    qs = []
    for c in range(N_CORES):
        qc = np.asarray(res.results[c]["q_out"])  # [128, RT]
        qs.append(qc.T.reshape(-1))  # row = rt*128 + p order
    q_all = np.concatenate(qs).astype(np.float64)  # row = c*512 + rt*128 + p
    return q_all, res


def kernel(**inputs: np.ndarray) -> np.ndarray:
    q_all, _ = run_device_part(inputs)
    loss = SCALE * np.mean((q_all - float(M)) / 2.0)
    return np.float32(loss)


# revision 9
# speedup vs baseline: 1.0139x; 1.0139x over previous
"""Diversity7 loss kernel for Trainium2 (8 NeuronCores, Bass/Tile).

Math (per batch row b):
  p_m   = softmax(x_m / T)                          m = 0..6, C = 1000 classes
  v_m   = (p_m - mean(p_m)) / ||p_m - mean(p_m)||   (mean(p_m) = 1/C exactly)
  q_b   = || sum_m v_m ||^2
  loss  = SCALE * mean_b((q_b - M) / 2)

Device-side restructuring (all f32):
  e    = exp(x/T)            (ACT pass, accum_out gives Se = sum e)
  S2   = sum_c e^2           (uncentered; split between ACT Square+accum and
                              DVE affine_mul_reduce(scale=1,bias=0) so neither
                              engine saturates, and no Se->dev2 serialization)
  dev2 = S2 - Se^2/C         (f32 smalls; cancellation costs ~1e-3 rel on
                              dev2 which is far inside the 2e-2 budget)
  g    = rsqrt(dev2) via magic-constant seed + 2 Newton steps (DVE-only)
  h    = -g*Se/C;  s = sum_m (g_m*e_m + h_m)   (fused affine_then_add chain)
  q    = sum_c s^2           (DVE affine_mul_reduce early row-tiles, ACT
                              Square+accum late ones where ACT idles)
Host finishes in f64: loss = SCALE * mean((q-7)/2).

Sharding: data-parallel over batch. 8 cores x 512 rows; each core sees
[512,1000] slices of the 7 logit tensors and emits q for its rows as [128,4]
(partition p, row-tile rt) -> global row = core*512 + rt*128 + p.
`targets` is accepted and ignored (unused by the reference loss).
"""

import sys

import numpy as np

if "/opt/trn_rl_repo" not in sys.path:
    sys.path.insert(0, "/opt/trn_rl_repo")

import concourse.bass as bass
import concourse.tile as tile
from concourse import bacc, mybir
from concourse.bass_utils import run_bass_kernel_spmd


def _patch_act_tables() -> None:
    """Make Exp/Square resolve only via natural_log_exp_and_others so the
    kernel needs exactly one ACT table load (the default first-fit choice
    thrashes table sets, ~1.3us per switch)."""
    import concourse.hw_specs as hw_specs

    if getattr(hw_specs, "_diversity7_patched", False):
        return
    orig = hw_specs.get_activation_tables

    def patched(module_arch):
        tables = orig(module_arch)
        keep = "natural_log_exp_and_others"
        if keep in tables:
            only = {
                mybir.ActivationFunctionType.Exp,
                mybir.ActivationFunctionType.Ln,
                mybir.ActivationFunctionType.Square,
            }
            for name, funcs in tables.items():
                if name != keep:
                    funcs -= only
        return tables

    hw_specs.get_activation_tables = patched
    bacc.get_activation_tables = patched
    hw_specs._diversity7_patched = True


T = 20.0
SCALE = 0.3
C = 1000
M = 7
N_CORES = 8
ROWS_PER_CORE = 512
RT = ROWS_PER_CORE // 128  # row-tiles per core

# Engine balance: how many of the 7 S2 (sum e^2) units run on ACT (Square+
# accum) vs DVE (affine_mul_reduce).  First row-tile keeps ACT light so the
# pipeline fills fast.
ACT_S2_COUNT = (2, 3, 3, 3)
# q = sum s^2 placement per row-tile: DVE early (ACT still busy with exps),
# ACT late (ACT idles during the tail while DVE finishes chains).
Q_ON_ACT = (False, False, True, False)

F32 = mybir.dt.float32
I32 = mybir.dt.int32
AF = mybir.ActivationFunctionType
ALU = mybir.AluOpType


def _build_program() -> bass.Bass:
    _patch_act_tables()
    nc = bacc.Bacc()
    xs = [
        nc.declare_dram_parameter(f"x{m}", [ROWS_PER_CORE, C], F32, isOutput=False)
        for m in range(M)
    ]
    q_out = nc.declare_dram_parameter("q_out", [128, RT], F32, isOutput=True)

    with tile.TileContext(nc) as tc:
        with (
            tc.tile_pool(name="xp", bufs=24) as xp,
            tc.tile_pool(name="ep", bufs=1) as ep,
            tc.tile_pool(name="trp", bufs=3) as trp,
            tc.tile_pool(name="sp", bufs=3) as sp,
            tc.tile_pool(name="smp", bufs=1) as smp,
            tc.tile_pool(name="qp", bufs=1) as qp,
        ):
            q = qp.tile([128, RT], F32)
            # Constant [128,M] tiles so the whole smalls block runs as GpSimd
            # tensor_tensor ops (GPS ts is ~1.4us but GPS tt is ~240ns; DVE
            # is the bottleneck engine so smalls move off it entirely).
            magic_t = smp.tile([128, M], I32, tag="magic_t")
            nc.vector.memset(magic_t[:], 0x5F3759DF)
            one_t = smp.tile([128, M], I32, tag="one_t")
            nc.vector.memset(one_t[:], 1)

            def phase1(rt: int):
                """DMA + exp(+Se accum) + S2 units for row-tile rt."""
                n_act = ACT_S2_COUNT[rt]
                Se = smp.tile([128, M], F32, tag="Se", bufs=2, name=f"Se{rt}")
                S2 = smp.tile([128, M], F32, tag="S2", bufs=2, name=f"S2{rt}")
                es: list[bass.AP] = []
                for m in range(M):
                    k = rt * M + m
                    x = xp.tile([128, C], F32, tag="x", name=f"x_{k}")
                    nc.sync.dma_start(x[:], xs[m][rt * 128 : (rt + 1) * 128, :])
                    e = ep.tile([128, C], F32, tag=f"e{m}", bufs=2, name=f"e_{k}")
                    nc.scalar.activation(
                        e[:], x[:], AF.Exp, bias=0.0, scale=1.0 / T,
                        accum_out=Se[:, m : m + 1],
                    )
                    trash = trp.tile([128, C], F32, tag="trash", name=f"tr_{k}")
                    if m >= M - n_act:
                        # S2 = sum e^2 on ACT (Square, no bias -> no Se dep)
                        nc.scalar.activation(
                            trash[:], e[:], AF.Square, bias=0.0, scale=1.0,
                            accum_out=S2[:, m : m + 1],
                        )
                    else:
                        # S2 = sum (1*e+0)*e on DVE
                        nc.vector.affine_mul_reduce(
                            out=trash[:], accum_out=S2[:, m : m + 1],
                            in0=e[:], in1=e[:], scale=1.0, bias=0.0,
                        )
                    es.append(e)
                return Se, S2, es

            def phase2_3(rt: int, Se, S2, es: list[bass.AP]):
                """Smalls (dev2, rsqrt, g, h) on DVE, then chain + q."""
                def dtile(nm, dt=F32, b=2):
                    return smp.tile([128, M], dt, tag=f"sm_{nm}",
                                    bufs=b, name=f"sm_{nm}_{rt}")
                # dev2 = S2 - Se^2/C  (2 ops)
                Se2 = dtile("Se2")
                nc.vector.tensor_tensor(Se2[:], Se[:], Se[:], ALU.mult)
                dev2 = dtile("dev2")
                nc.vector.scalar_tensor_tensor(
                    dev2[:], Se2[:], -1.0 / C, S2[:], op0=ALU.mult, op1=ALU.add,
                )
                # rsqrt via magic seed + 2 Newton steps (seed rel err 3.4e-3,
                # after 2 steps ~4e-10 -- exact at f32 for this tolerance)
                half_i = dtile("hi", I32)
                nc.vector.tensor_scalar(
                    half_i[:], dev2[:].bitcast(I32), one_t[:, 0:1], None,
                    op0=ALU.logical_shift_right,
                )
                seed_i = dtile("si", I32)
                nc.vector.tensor_tensor(seed_i[:], magic_t[:], half_i[:],
                                        ALU.subtract)
                y = seed_i[:].bitcast(F32)
                for it in range(2):
                    ysq = dtile(f"ysq{it}")
                    nc.vector.tensor_tensor(ysq[:], y, y, ALU.mult)
                    zy = dtile(f"zy{it}")
                    nc.vector.tensor_tensor(zy[:], dev2[:], ysq[:], ALU.mult)
                    nrc = dtile(f"nrc{it}")
                    nc.vector.tensor_scalar(
                        nrc[:], zy[:], -0.5, 1.5, op0=ALU.mult, op1=ALU.add
                    )
                    yn = dtile(f"invr{it}")
                    nc.vector.tensor_tensor(yn[:], y, nrc[:], ALU.mult)
                    y = yn[:]
                g = y
                hs = dtile("hs")
                nc.vector.tensor_tensor(hs[:], Se[:], y, ALU.mult)
                h = dtile("h")
                nc.vector.tensor_scalar_mul(h[:], hs[:], -1.0 / C)

                # s = sum_m (g_m*e_m + h_m) via fused affine_then_add chain
                s_prev = None
                for m in range(M):
                    s_new = sp.tile([128, C], F32, tag="s", name=f"s{rt}_{m}")
                    if m == 0:
                        nc.vector.tensor_scalar(
                            s_new[:], es[0][:], g[0:128, 0:1], h[:, 0:1],
                            op0=ALU.mult, op1=ALU.add,
                        )
                    else:
                        nc.vector.affine_then_add(
                            s_new[:], es[m][:], s_prev[:], g[0:128, m : m + 1],
                            h[:, m : m + 1],
                        )
                    s_prev = s_new
                trash2 = trp.tile([128, C], F32, tag="trash", name=f"tr2_{rt}")
                if Q_ON_ACT[rt]:
                    nc.scalar.activation(
                        trash2[:], s_prev[:], AF.Square, bias=0.0, scale=1.0,
                        accum_out=q[:, rt : rt + 1],
                    )
                else:
                    nc.vector.affine_mul_reduce(
                        out=trash2[:], accum_out=q[:, rt : rt + 1],
                        in0=s_prev[:], in1=s_prev[:], scale=1.0, bias=0.0,
                    )

            # Software pipeline: emit row-tile rt+1's phase 1 BEFORE row-tile
            # rt's scalar math + chain, so the (FIFO) engine queues always
            # have ready phase-1 work at row-tile boundaries.
            DEPTH = 1
            pending = []
            for rt in range(RT):
                pending.append((rt, *phase1(rt)))
                if len(pending) > DEPTH:
                    phase2_3(*pending.pop(0))
            for args in pending:
                phase2_3(*args)
            nc.sync.dma_start(q_out[:], q[:])
    return nc


_NC_CACHE: bass.Bass | None = None


def _get_program() -> bass.Bass:
    global _NC_CACHE
    if _NC_CACHE is None:
        nc = _build_program()
        nc.finalize()
        _NC_CACHE = nc
    return _NC_CACHE


def run_device_part(inputs: dict[str, np.ndarray], **run_kwargs):
    """Run the bass kernel; returns (q_all [4096] f64 row-major, results)."""
    nc = _get_program()
    core_ids = list(range(N_CORES))
    in_maps = []
    for c in range(N_CORES):
        lo, hi = c * ROWS_PER_CORE, (c + 1) * ROWS_PER_CORE
        im = {
            f"x{m}": np.ascontiguousarray(
                inputs[f"outputs{m + 1}"][lo:hi], dtype=np.float32
            )
            for m in range(M)
        }
        in_maps.append(im)
    res = run_bass_kernel_spmd(nc, in_maps, core_ids, **run_kwargs)
    qs = []
    for c in range(N_CORES):
        qc = np.asarray(res.results[c]["q_out"])  # [128, RT]
        qs.append(qc.T.reshape(-1))  # row = rt*128 + p order
    q_all = np.concatenate(qs).astype(np.float64)  # row = c*512 + rt*128 + p
    return q_all, res


def kernel(**inputs: np.ndarray) -> np.ndarray:
    q_all, _ = run_device_part(inputs)
    loss = SCALE * np.mean((q_all - float(M)) / 2.0)
    return np.float32(loss)


# revision 11
# speedup vs baseline: 1.0482x; 1.0339x over previous
"""Diversity7 loss kernel for Trainium2 (8 NeuronCores, Bass/Tile).

Math (per batch row b):
  p_m   = softmax(x_m / T)                          m = 0..6, C = 1000 classes
  v_m   = (p_m - mean(p_m)) / ||p_m - mean(p_m)||   (mean(p_m) = 1/C exactly)
  q_b   = || sum_m v_m ||^2
  loss  = SCALE * mean_b((q_b - M) / 2)

Device-side restructuring (all f32):
  e    = exp(x/T)            (ACT pass, accum_out gives Se = sum e)
  S2   = sum_c e^2           (uncentered; split between ACT Square+accum and
                              DVE affine_mul_reduce(scale=1,bias=0) so neither
                              engine saturates, and no Se->dev2 serialization)
  dev2 = S2 - Se^2/C         (f32 smalls; cancellation costs ~1e-3 rel on
                              dev2 which is far inside the 2e-2 budget)
  g    = rsqrt(dev2) via magic-constant seed + 2 Newton steps (DVE-only)
  h    = -g*Se/C;  s = sum_m (g_m*e_m + h_m)   (fused affine_then_add chain)
  q    = sum_c s^2           (DVE affine_mul_reduce early row-tiles, ACT
                              Square+accum late ones where ACT idles)
Host finishes in f64: loss = SCALE * mean((q-7)/2).

Sharding: data-parallel over batch. 8 cores x 512 rows; each core sees
[512,1000] slices of the 7 logit tensors and emits q for its rows as [128,4]
(partition p, row-tile rt) -> global row = core*512 + rt*128 + p.
`targets` is accepted and ignored (unused by the reference loss).
"""

import sys

import numpy as np

if "/opt/trn_rl_repo" not in sys.path:
    sys.path.insert(0, "/opt/trn_rl_repo")

import concourse.bass as bass
import concourse.tile as tile
from concourse import bacc, mybir
from concourse.bass_utils import run_bass_kernel_spmd


def _patch_act_tables() -> None:
    """Make Exp/Square resolve only via natural_log_exp_and_others so the
    kernel needs exactly one ACT table load (the default first-fit choice
    thrashes table sets, ~1.3us per switch)."""
    import concourse.hw_specs as hw_specs

    if getattr(hw_specs, "_diversity7_patched", False):
        return
    orig = hw_specs.get_activation_tables

    def patched(module_arch):
        tables = orig(module_arch)
        keep = "natural_log_exp_and_others"
        if keep in tables:
            only = {
                mybir.ActivationFunctionType.Exp,
                mybir.ActivationFunctionType.Ln,
                mybir.ActivationFunctionType.Square,
            }
            for name, funcs in tables.items():
                if name != keep:
                    funcs -= only
        return tables

    hw_specs.get_activation_tables = patched
    bacc.get_activation_tables = patched
    hw_specs._diversity7_patched = True


T = 20.0
SCALE = 0.3
C = 1000
M = 7
N_CORES = 8
ROWS_PER_CORE = 512
RT = ROWS_PER_CORE // 128  # row-tiles per core

# Engine balance: how many of the 7 S2 (sum e^2) units run on ACT (Square+
# accum) vs DVE (affine_mul_reduce).  First row-tile keeps ACT light so the
# pipeline fills fast.
ACT_S2_COUNT = (2, 3, 3, 3)
# q = sum s^2 placement per row-tile: DVE early (ACT still busy with exps),
# ACT late (ACT idles during the tail while DVE finishes chains).
Q_ON_ACT = (False, False, True, False)

F32 = mybir.dt.float32
I32 = mybir.dt.int32
AF = mybir.ActivationFunctionType
ALU = mybir.AluOpType


def _build_program() -> bass.Bass:
    _patch_act_tables()
    nc = bacc.Bacc()
    xs = [
        nc.declare_dram_parameter(f"x{m}", [ROWS_PER_CORE, C], F32, isOutput=False)
        for m in range(M)
    ]
    q_out = nc.declare_dram_parameter("q_out", [128, RT], F32, isOutput=True)

    with tile.TileContext(nc) as tc:
        with (
            tc.tile_pool(name="xp", bufs=24) as xp,
            tc.tile_pool(name="ep", bufs=1) as ep,
            tc.tile_pool(name="trp", bufs=3) as trp,
            tc.tile_pool(name="sp", bufs=3) as sp,
            tc.tile_pool(name="smp", bufs=1) as smp,
            tc.tile_pool(name="qp", bufs=1) as qp,
        ):
            q = qp.tile([128, RT], F32)
            # Constant [128,M] tiles so the whole smalls block runs as GpSimd
            # tensor_tensor ops (GPS ts is ~1.4us but GPS tt is ~240ns; DVE
            # is the bottleneck engine so smalls move off it entirely).
            magic_t = smp.tile([128, M], I32, tag="magic_t")
            nc.vector.memset(magic_t[:], 0x5F3759DF)
            one_t = smp.tile([128, M], I32, tag="one_t")
            nc.vector.memset(one_t[:], 1)

            def phase1(rt: int):
                """DMA + exp(+Se accum) + S2 units for row-tile rt."""
                n_act = ACT_S2_COUNT[rt]
                # ACT-side S2 models: late models normally (DVE's amr work
                # starts early); EARLY models for the last row-tile so the
                # tail's critical path is just exp6 -> smalls -> chain end.
                if rt == RT - 1:
                    act_s2 = set(range(n_act))
                else:
                    act_s2 = set(range(M - n_act, M))
                Se = smp.tile([128, M], F32, tag="Se", bufs=2, name=f"Se{rt}")
                S2 = smp.tile([128, M], F32, tag="S2", bufs=2, name=f"S2{rt}")
                es: list[bass.AP] = []
                for m in range(M):
                    k = rt * M + m
                    x = xp.tile([128, C], F32, tag="x", name=f"x_{k}")
                    nc.sync.dma_start(x[:], xs[m][rt * 128 : (rt + 1) * 128, :])
                    e = ep.tile([128, C], F32, tag=f"e{m}", bufs=2, name=f"e_{k}")
                    nc.scalar.activation(
                        e[:], x[:], AF.Exp, bias=0.0, scale=1.0 / T,
                        accum_out=Se[:, m : m + 1],
                    )
                    trash = trp.tile([128, C], F32, tag="trash", name=f"tr_{k}")
                    if m in act_s2:
                        # S2 = sum e^2 on ACT (Square, no bias -> no Se dep)
                        nc.scalar.activation(
                            trash[:], e[:], AF.Square, bias=0.0, scale=1.0,
                            accum_out=S2[:, m : m + 1],
                        )
                    else:
                        # S2 = sum (1*e+0)*e on DVE
                        nc.vector.affine_mul_reduce(
                            out=trash[:], accum_out=S2[:, m : m + 1],
                            in0=e[:], in1=e[:], scale=1.0, bias=0.0,
                        )
                    es.append(e)
                return Se, S2, es

            def smalls(rt: int, bat: int, Se, S2, lo: int, hi: int):
                """dev2 -> g,h for models [lo,hi) of row-tile rt (DVE)."""
                n = hi - lo
                sl = slice(lo, hi)

                def dtile(nm, dt=F32, b=2):
                    return smp.tile([128, n], dt, tag=f"sm{bat}_{nm}",
                                    bufs=b, name=f"sm{bat}_{nm}_{rt}")
                # dev2 = S2 - Se^2/C  (2 ops)
                Se2 = dtile("Se2")
                nc.vector.tensor_tensor(Se2[:], Se[:, sl], Se[:, sl], ALU.mult)
                dev2 = dtile("dev2")
                nc.vector.scalar_tensor_tensor(
                    dev2[:], Se2[:], -1.0 / C, S2[:, sl],
                    op0=ALU.mult, op1=ALU.add,
                )
                # rsqrt via magic seed + 2 Newton steps (seed rel err 3.4e-3,
                # after 2 steps ~4e-10 -- exact at f32 for this tolerance)
                half_i = dtile("hi", I32)
                nc.vector.tensor_scalar(
                    half_i[:], dev2[:].bitcast(I32), one_t[:, 0:1], None,
                    op0=ALU.logical_shift_right,
                )
                seed_i = dtile("si", I32)
                nc.vector.tensor_tensor(seed_i[:], magic_t[:, 0:n], half_i[:],
                                        ALU.subtract)
                y = seed_i[:].bitcast(F32)
                for it in range(2):
                    ysq = dtile(f"ysq{it}")
                    nc.vector.tensor_tensor(ysq[:], y, y, ALU.mult)
                    zy = dtile(f"zy{it}")
                    nc.vector.tensor_tensor(zy[:], dev2[:], ysq[:], ALU.mult)
                    nrc = dtile(f"nrc{it}")
                    nc.vector.tensor_scalar(
                        nrc[:], zy[:], -0.5, 1.5, op0=ALU.mult, op1=ALU.add
                    )
                    yn = dtile(f"invr{it}")
                    nc.vector.tensor_tensor(yn[:], y, nrc[:], ALU.mult)
                    y = yn[:]
                g = y
                hs = dtile("hs")
                nc.vector.tensor_tensor(hs[:], Se[:, sl], y, ALU.mult)
                h = dtile("h")
                nc.vector.tensor_scalar_mul(h[:], hs[:], -1.0 / C)
                return g, h

            def chain_part(rt: int, es, g, h, lo: int, hi: int, s_prev):
                """Extend s with models [lo,hi); g/h are [128, hi-lo]."""
                for m in range(lo, hi):
                    j = m - lo
                    s_new = sp.tile([128, C], F32, tag="s", name=f"s{rt}_{m}")
                    if s_prev is None:
                        nc.vector.tensor_scalar(
                            s_new[:], es[m][:], g[0:128, j : j + 1],
                            h[:, j : j + 1], op0=ALU.mult, op1=ALU.add,
                        )
                    else:
                        nc.vector.affine_then_add(
                            s_new[:], es[m][:], s_prev[:], g[0:128, j : j + 1],
                            h[:, j : j + 1],
                        )
                    s_prev = s_new
                return s_prev

            def phase2_3(rt: int, Se, S2, es: list[bass.AP]):
                """Smalls then chain + q.  The last row-tile runs in two
                batches so its chain starts before the last exps' accums
                land (shorter serial tail)."""
                if rt == RT - 1:
                    split = 5
                    g_a, h_a = smalls(rt, 0, Se, S2, 0, split)
                    s = chain_part(rt, es, g_a, h_a, 0, split, None)
                    g_b, h_b = smalls(rt, 1, Se, S2, split, M)
                    s = chain_part(rt, es, g_b, h_b, split, M, s)
                else:
                    g, h = smalls(rt, 0, Se, S2, 0, M)
                    s = chain_part(rt, es, g, h, 0, M, None)
                trash2 = trp.tile([128, C], F32, tag="trash", name=f"tr2_{rt}")
                if Q_ON_ACT[rt]:
                    nc.scalar.activation(
                        trash2[:], s[:], AF.Square, bias=0.0, scale=1.0,
                        accum_out=q[:, rt : rt + 1],
                    )
                else:
                    nc.vector.affine_mul_reduce(
                        out=trash2[:], accum_out=q[:, rt : rt + 1],
                        in0=s[:], in1=s[:], scale=1.0, bias=0.0,
                    )

            # Software pipeline: emit row-tile rt+1's phase 1 BEFORE row-tile
            # rt's scalar math + chain, so the (FIFO) engine queues always
            # have ready phase-1 work at row-tile boundaries.
            DEPTH = 1
            pending = []
            for rt in range(RT):
                pending.append((rt, *phase1(rt)))
                if len(pending) > DEPTH:
                    phase2_3(*pending.pop(0))
            for args in pending:
                phase2_3(*args)
            nc.sync.dma_start(q_out[:], q[:])
    return nc


_NC_CACHE: bass.Bass | None = None


def _get_program() -> bass.Bass:
    global _NC_CACHE
    if _NC_CACHE is None:
        nc = _build_program()
        nc.finalize()
        _NC_CACHE = nc
    return _NC_CACHE


def run_device_part(inputs: dict[str, np.ndarray], **run_kwargs):
    """Run the bass kernel; returns (q_all [4096] f64 row-major, results)."""
    nc = _get_program()
    core_ids = list(range(N_CORES))
    in_maps = []
    for c in range(N_CORES):
        lo, hi = c * ROWS_PER_CORE, (c + 1) * ROWS_PER_CORE
        im = {
            f"x{m}": np.ascontiguousarray(
                inputs[f"outputs{m + 1}"][lo:hi], dtype=np.float32
            )
            for m in range(M)
        }
        in_maps.append(im)
    res = run_bass_kernel_spmd(nc, in_maps, core_ids, **run_kwargs)
    qs = []
    for c in range(N_CORES):
        qc = np.asarray(res.results[c]["q_out"])  # [128, RT]
        qs.append(qc.T.reshape(-1))  # row = rt*128 + p order
    q_all = np.concatenate(qs).astype(np.float64)  # row = c*512 + rt*128 + p
    return q_all, res


def kernel(**inputs: np.ndarray) -> np.ndarray:
    q_all, _ = run_device_part(inputs)
    loss = SCALE * np.mean((q_all - float(M)) / 2.0)
    return np.float32(loss)
